# revision 13
# baseline (speedup 1.0000x reference)
"""Trainium2 Bass kernel for nn_CandidateFilterModel (segment_reduce).

Strategy (8 cores, S-column sharding for the heavy phases, pair sharding for the tail):
  - Core k owns sequence-column slice s_k = [256k, 256k+256).
  - Phase 1: entity aggregation.
      ent_emb^T = log(OH_emb-matmul of exp(seq[mention_idx]))   (replicated, bf16)
      ent_att (local s-slice) = OH_mean-matmul of gathered attention rows (fp8)
      One-hot slabs that are all-zero (entity_ids is sorted, so each mention
      tile only spans ~32 entities) are skipped entirely.
  - Phase 2: pair products. For all 2048 pairs: gather ent_att rows of head/tail
      entity (4KB fp8 rows, indirect DMA), multiply (fp8 in, bf16 out), one DVE
      add folds 16 heads -> 8, then PE transpose-ACCUMULATE matmuls (x identity)
      fold the remaining 8 head-blocks while transposing -> raw^T in PSUM.
  - Phase 3: TWO AllToAlls (even pair-tiles = first 128 pairs of each dest
      core, then odd) redistribute raw^T so core k holds raw^T[:, P_k].
  - Phases 4-6 (per pair-half): contexts via seq^T-matmul, normalize, z_s/z_o
      via (ent_emb @ W)-then-gather one-hot matmuls + W_ctx matmuls + tanh,
      bilinear via W_bil matmuls + elementwise + ones-reduction matmul.
Host pre-casts: attention fp8 e4m3 (quantization error largely cancels in the
pair_att normalization), seq/weights bf16. DMA queues: gpsimd = indirect
gathers + collectives, sync = small loads/staging/paT, scalar = weight loads.
PSUM->SBUF copies in the tail ride the scalar engine to keep DVE free.
"""
import sys
import types
import numpy as np

S, H, HEADS = 2048, 1024, 16
E, NM, P = 256, 1024, 2048
PH = 1024
NC = 8
SL = S // NC          # 256 s-columns per core
PL = P // NC          # 256 pairs per core
NMT = NM // 128       # 8 mention tiles
NPT = P // 128        # 16 pair tiles
HS = HEADS * SL       # 4096 = width of per-core ent_att rows

_CACHE = {}

# feature flags (bisectable); read at build time and folded into the cache key
FLAGS = {
    "PH1_MERGED": False,   # 2-col merged gathers: CRASHES HW (worker hangup)
    "SCALAR_F8": True,     # entA PSUM->fp8 copies on scalar engine
    "PE_GATHER": True,     # PE one-hot head-gather for even pair tiles
    "CAST_GATHER": True,   # fp8->bf16 cast during th/tt gathers
}


def _ensure_axon_profile_hook():
    """bass_utils' trace path imports antenv.axon_hooks, absent in this image."""
    if 'antenv.axon_hooks' in sys.modules:
        return
    try:
        import antenv.axon_hooks  # noqa: F401
        return
    except ImportError:
        pass
    mod = types.ModuleType('antenv.axon_hooks')
    holder = [None]
    mod.set_axon_ntff_profile_hook = lambda h: holder.__setitem__(0, h)
    mod.get_axon_ntff_profile_hook = lambda: holder[0]
    sys.modules['antenv.axon_hooks'] = mod
    try:
        from trn_agent_boot.trn_boot import _ntff_profile_via_ctypes
        hook = _ntff_profile_via_ctypes('/opt/axon/libaxon_pjrt.so')
        if hook is not None:
            mod.set_axon_ntff_profile_hook(hook)
    except Exception:
        pass


def _build(mt_ets, debug=False):
    """mt_ets: per mention-tile, tuple of entity-128-halves it touches."""
    import concourse.bass as bass
    import concourse.bacc as bacc
    import concourse.tile as tile
    from concourse import mybir
    from concourse.masks import make_identity

    F32 = mybir.dt.float32
    BF16 = mybir.dt.bfloat16
    F8 = mybir.dt.float8e4
    I32 = mybir.dt.int32
    AF = mybir.ActivationFunctionType
    OP = mybir.AluOpType

    nc = bacc.Bacc(num_devices=NC)

    # ---------------- inputs ----------------
    att_k = nc.declare_dram_parameter("att_k", [S, HS], F8, isOutput=False)
    seqb = nc.declare_dram_parameter("seqb", [S, H], BF16, isOutput=False)
    m_off = nc.declare_dram_parameter("m_off", [128, NMT], I32, isOutput=False)
    p_off = nc.declare_dram_parameter("p_off", [128, 2 * NPT], I32, isOutput=False)
    ohe = nc.declare_dram_parameter("ohe", [NM, E], BF16, isOutput=False)
    ohm = nc.declare_dram_parameter("ohm", [NM, E], F8, isOutput=False)
    has0r = nc.declare_dram_parameter("has0r", [1, E], F32, isOutput=False)
    ohg = nc.declare_dram_parameter("ohg", [E, 8 * 128], F8, isOutput=False)
    ohh_k = nc.declare_dram_parameter("ohh_k", [E, PL], BF16, isOutput=False)
    oht_k = nc.declare_dram_parameter("oht_k", [E, PL], BF16, isOutput=False)
    w_head = nc.declare_dram_parameter("w_head", [H, PH], BF16, isOutput=False)
    w_tail = nc.declare_dram_parameter("w_tail", [H, PH], BF16, isOutput=False)
    w_ctx = nc.declare_dram_parameter("w_ctx", [H, PH], BF16, isOutput=False)
    w_bil = nc.declare_dram_parameter("w_bil", [PH, PH], BF16, isOutput=False)
    b_head = nc.declare_dram_parameter("b_head", [128, PH // 128], F32, isOutput=False)
    b_tail = nc.declare_dram_parameter("b_tail", [128, PH // 128], F32, isOutput=False)
    b_bil = nc.declare_dram_parameter("b_bil", [1, 1], F32, isOutput=False)
    out = nc.declare_dram_parameter("out", [1, PL], F32, isOutput=True)

    dbg = {}
    if debug:
        dbg["ent_embT"] = nc.declare_dram_parameter("d_ent_embT", [H, E], BF16, isOutput=True)
        dbg["entA"] = nc.declare_dram_parameter("d_entA", [E, HS], BF16, isOutput=True)
        dbg["rawT"] = nc.declare_dram_parameter("d_rawT", [128, 2 * NPT * 128], BF16, isOutput=True)
        dbg["ctxuT"] = nc.declare_dram_parameter("d_ctxuT", [H, PL], BF16, isOutput=True)
        dbg["zrow"] = nc.declare_dram_parameter("d_zrow", [1, PL], F32, isOutput=True)
        dbg["zsT"] = nc.declare_dram_parameter("d_zsT", [PH, PL], BF16, isOutput=True)

    # internal DRAM
    entA_dram = nc.dram_tensor("entA_dram", [E, HS], F8)
    a2a_in = [nc.dram_tensor(f"a2a{h}_in", [NC, SL, 128], BF16) for h in range(2)]
    a2a_out = [nc.dram_tensor(f"a2a{h}_out", [NC, SL, 128], BF16) for h in range(2)]

    et_mts = {0: [mt for mt in range(NMT) if 0 in mt_ets[mt]],
              1: [mt for mt in range(NMT) if 1 in mt_ets[mt]]}

    with tile.TileContext(nc) as tc:
        with tc.tile_pool(name="singles", bufs=1) as singles, \
             tc.tile_pool(name="wpool", bufs=1) as wpool:
            # ---------------- phase 0: small loads (sync queue) ----------------
            m_off_t = singles.tile([128, NMT], I32)
            nc.sync.dma_start(out=m_off_t, in_=m_off[:, :])
            p_off_t = singles.tile([128, 2 * NPT], I32)
            nc.sync.dma_start(out=p_off_t, in_=p_off[:, :])
            ohg_t = singles.tile([128, 2, 8, 128], F8)
            nc.sync.dma_start(out=ohg_t, in_=ohg.rearrange("(t p) (i q) -> p t i q", p=128, q=128))
            ohh_t = singles.tile([128, 2, PL], BF16)
            nc.sync.dma_start(out=ohh_t, in_=ohh_k.rearrange("(t p) q -> p t q", p=128))
            oht_t = singles.tile([128, 2, PL], BF16)
            nc.sync.dma_start(out=oht_t, in_=oht_k.rearrange("(t p) q -> p t q", p=128))
            bhs_t = singles.tile([128, PH // 128], F32)
            nc.sync.dma_start(out=bhs_t, in_=b_head[:, :])
            bts_t = singles.tile([128, PH // 128], F32)
            nc.sync.dma_start(out=bts_t, in_=b_tail[:, :])
            bbil_t = singles.tile([1, 1], F32)
            nc.sync.dma_start(out=bbil_t, in_=b_bil[:, :])
            ident = singles.tile([128, 128], BF16)
            make_identity(nc, ident[:, :])
            # warm activation tables; Exp last = first real user
            warm = singles.tile([1, 8], F32)
            nc.vector.memset(warm[:, :], 0.0)
            nc.scalar.activation(out=warm[:, :], in_=warm[:, :], func=AF.Tanh)
            nc.scalar.activation(out=warm[:, :], in_=warm[:, :], func=AF.Ln)
            nc.scalar.activation(out=warm[:, :], in_=warm[:, :], func=AF.Exp)
            ones_col = singles.tile([128, 1], BF16)
            nc.vector.memset(ones_col[:, :], 1.0)
            ones_row = singles.tile([1, 128], BF16)
            nc.vector.memset(ones_row[:, :], 1.0)

            entTe = singles.tile([128, H // 128, E], BF16)  # ent_emb^T [hcol-part, hc, e]
            entA_sb = singles.tile([128, 2, HS], F8)        # ent_att fp8 [e-part, et, (h s)]
            rawT = singles.tile([128, 2, NPT, 128], BF16)   # [s-part, sh, pt, p-row]
            paT = singles.tile([128, S // 128, PL], BF16)   # raw^T for my pairs, all s
            ucb = singles.tile([128, H // 128, PL], BF16)   # contexts^T (unnormalized)
            ctxT = singles.tile([128, H // 128, 128], BF16)
            zsT = singles.tile([128, PH // 128, 128], BF16)
            zoT = singles.tile([128, PH // 128, 128], BF16)
            dbg_zs = singles.tile([128, PH // 128, PL], BF16) if debug else None
            EWh = singles.tile([128, 2, PH], BF16)          # ent_emb @ W_head [e-part, et, PH]
            EWt = singles.tile([128, 2, PH], BF16)
            zrow = singles.tile([1, PL], F32)
            zrec = singles.tile([128, PL], BF16)
            zrec_b = singles.tile([1, PL], BF16)
            lg_sb = singles.tile([1, PL], F32)

            # weight tiles; loads are emitted after the critical ph1 gathers
            # so they don't steal HBM bandwidth from them.
            whb = wpool.tile([128, H // 128, PH], BF16)
            wtb = wpool.tile([128, H // 128, PH], BF16)
            wcb = wpool.tile([128, H // 128, PH], BF16)
            wbb = wpool.tile([128, PH // 128, PH], BF16)
            seqx = wpool.tile([128, S // 128, H], BF16)

            # ---------------- phase 1: gathers + entity aggregation ----------------
            with tc.tile_pool(name="p1", bufs=1) as p1:
                ohe_t = p1.tile([128, NMT, E], BF16)
                nc.sync.dma_start(out=ohe_t, in_=ohe.rearrange("(t p) e -> p t e", p=128))
                ohm_t = p1.tile([128, NMT, E], F8)
                nc.sync.dma_start(out=ohm_t, in_=ohm.rearrange("(t p) e -> p t e", p=128))
                has0b = p1.tile([128, E], F32)
                nc.sync.dma_start(out=has0b, in_=has0r[:, :].to_broadcast([128, E]))
                ag = []
                ev = []
                if FLAGS["PH1_MERGED"]:
                    for mg in range(NMT // 2):
                        g = p1.tile([128, 2, HS], F8, tag=f"ag{mg}")
                        nc.gpsimd.indirect_dma_start(
                            out=g[:, :, :], out_offset=None, in_=att_k[:, :],
                            in_offset=bass.IndirectOffsetOnAxis(
                                ap=m_off_t[:, 2 * mg:2 * mg + 2], axis=0))
                        ag.extend([g[:, 0, :], g[:, 1, :]])
                    for mg in range(NMT // 2):
                        vg = p1.tile([128, 2, H], BF16, tag=f"vg{mg}")
                        nc.gpsimd.indirect_dma_start(
                            out=vg[:, :, :], out_offset=None, in_=seqb[:, :],
                            in_offset=bass.IndirectOffsetOnAxis(
                                ap=m_off_t[:, 2 * mg:2 * mg + 2], axis=0))
                        nc.scalar.activation(out=vg[:, :, :], in_=vg[:, :, :],
                                             func=AF.Exp)
                        ev.extend([vg[:, 0, :], vg[:, 1, :]])
                else:
                    for mt in range(NMT):
                        g = p1.tile([128, HS], F8, tag=f"ags{mt}")
                        nc.gpsimd.indirect_dma_start(
                            out=g[:, :], out_offset=None, in_=att_k[:, :],
                            in_offset=bass.IndirectOffsetOnAxis(
                                ap=m_off_t[:, mt:mt + 1], axis=0))
                        ag.append(g)
                    for mt in range(NMT):
                        vg = p1.tile([128, H], BF16, tag=f"vgs{mt}")
                        nc.gpsimd.indirect_dma_start(
                            out=vg[:, :], out_offset=None, in_=seqb[:, :],
                            in_offset=bass.IndirectOffsetOnAxis(
                                ap=m_off_t[:, mt:mt + 1], axis=0))
                        nc.scalar.activation(out=vg[:, :], in_=vg[:, :], func=AF.Exp)
                        ev.append(vg)

                # big loads (scalar HWDGE queue): whb/wtb first (EW needs them),
                # emitted after the ph1-critical gathers
                nc.scalar.dma_start(out=whb, in_=w_head.rearrange("(t p) n -> p t n", p=128))
                nc.scalar.dma_start(out=wtb, in_=w_tail.rearrange("(t p) n -> p t n", p=128))
                nc.scalar.dma_start(out=wcb, in_=w_ctx.rearrange("(t p) n -> p t n", p=128))
                nc.scalar.dma_start(out=seqx, in_=seqb.rearrange("(t p) h -> p t h", p=128))
                nc.scalar.dma_start(out=wbb, in_=w_bil.rearrange("(t p) n -> p t n", p=128))

                # heads in 2 groups of 8 (PSUM capacity); both entity halves
                # accumulate concurrently; all-zero (mt, et) slabs are skipped.
                with tc.tile_pool(name="ps_a", bufs=1, space="PSUM") as ps_a:
                    for hg in range(2):
                        pas0 = ps_a.tile([128, 8 * SL], F32, space="PSUM", tag="agg0")
                        pas1 = ps_a.tile([128, 8 * SL], F32, space="PSUM", tag="agg1")
                        pas = {0: pas0, 1: pas1}
                        for et in range(2):
                            if not et_mts[et]:
                                nc.vector.memset(pas[et][:, :], 0.0)
                        for mt in range(NMT):
                            for et in mt_ets[mt]:
                                for nch in range(4):  # 2048 = 4 x 512
                                    nc.tensor.matmul(
                                        pas[et][:, nch * 512:(nch + 1) * 512],
                                        ohm_t[:, mt, et * 128:(et + 1) * 128],
                                        ag[mt][:, hg * 2048 + nch * 512:
                                               hg * 2048 + (nch + 1) * 512],
                                        start=(mt == et_mts[et][0]),
                                        stop=(mt == et_mts[et][-1]))
                        for et in range(2):
                            if FLAGS["SCALAR_F8"]:
                                nc.scalar.copy(
                                    out=entA_sb[:, et, hg * 2048:(hg + 1) * 2048],
                                    in_=pas[et][:, :])
                            else:
                                nc.vector.tensor_copy(
                                    out=entA_sb[:, et, hg * 2048:(hg + 1) * 2048],
                                    in_=pas[et][:, :])
                            nc.sync.dma_start(
                                out=entA_dram.rearrange("(t p) w -> p t w", p=128)[
                                    :, et, hg * 2048:(hg + 1) * 2048],
                                in_=entA_sb[:, et, hg * 2048:(hg + 1) * 2048])
                            if debug:
                                eAb = p1.tile([128, 8 * SL], BF16, tag="entA_dbg")
                                nc.vector.tensor_copy(out=eAb[:, :], in_=pas[et][:, :])
                                nc.sync.dma_start(
                                    out=dbg["entA"].rearrange("(t p) w -> p t w", p=128)[
                                        :, et, hg * 2048:(hg + 1) * 2048],
                                    in_=eAb[:, :])

                # ---------------- phase 1b: logsumexp matmuls ----------------
                with tc.tile_pool(name="ps_s", bufs=2, space="PSUM") as ps_s:
                    for hc in range(H // 128):
                        sp = ps_s.tile([128, E], F32, space="PSUM", tag="sums")
                        for et in range(2):
                            mts = et_mts[et]
                            if not mts:
                                nc.vector.memset(sp[:, et * 128:(et + 1) * 128], 0.0)
                                continue
                            for mt in mts:
                                nc.tensor.matmul(
                                    sp[:, et * 128:(et + 1) * 128],
                                    ev[mt][:, hc * 128:(hc + 1) * 128],
                                    ohe_t[:, mt, et * 128:(et + 1) * 128],
                                    start=(mt == mts[0]), stop=(mt == mts[-1]))
                        nc.vector.tensor_tensor(out=sp[:, :], in0=sp[:, :],
                                                in1=has0b[:, :], op=OP.add)
                        nc.scalar.activation(out=entTe[:, hc, :], in_=sp[:, :], func=AF.Ln)
            if debug:
                nc.sync.dma_start(
                    out=dbg["ent_embT"].rearrange("(t p) e -> p t e", p=128), in_=entTe)

            # ---------------- phase 2: pair products ----------------
            # evens (tiles 0,2,..,14 = first 128 pairs of each dest core) first
            # so AllToAll #A can fire while the odds still stream. Gathers cast
            # fp8->bf16 in the DMA so DVE runs in 2x mode. Head rows of the
            # even tiles come from PE one-hot matmuls out of SBUF (pairs are
            # sorted by head, Q7 descgen is the phase bottleneck).
            GDT = BF16 if FLAGS["CAST_GATHER"] else F8

            def pair_tile(pt, pe_head, pg, pg1, prod, ps_r, psg):
                pe_head = pe_head and FLAGS["PE_GATHER"]
                if pe_head:
                    i = pt // 2
                    th = pg.tile([128, HS], BF16, tag="thp")
                    for ch in range(8):
                        gp = psg.tile([128, 512], F32, space="PSUM", tag="gp")
                        for et in range(2):
                            nc.tensor.matmul(
                                gp[:, :], ohg_t[:, et, i, :],
                                entA_sb[:, et, ch * 512:(ch + 1) * 512],
                                start=(et == 0), stop=(et == 1))
                        nc.scalar.copy(out=th[:, ch * 512:(ch + 1) * 512], in_=gp[:, :])
                else:
                    th = pg1.tile([128, HS], GDT, tag="th")
                    nc.gpsimd.indirect_dma_start(
                        out=th[:, :], out_offset=None, in_=entA_dram[:, :],
                        in_offset=bass.IndirectOffsetOnAxis(
                            ap=p_off_t[:, 2 * pt:2 * pt + 1], axis=0))
                tt = pg.tile([128, HS], GDT, tag="tt")
                nc.gpsimd.indirect_dma_start(
                    out=tt[:, :], out_offset=None, in_=entA_dram[:, :],
                    in_offset=bass.IndirectOffsetOnAxis(
                        ap=p_off_t[:, 2 * pt + 1:2 * pt + 2], axis=0))
                pr = prod.tile([128, HS], BF16, tag="pr")
                nc.vector.tensor_tensor(out=pr[:, :], in0=th[:, :], in1=tt[:, :],
                                        op=OP.mult)
                # fold 16 heads -> 8 on DVE; remaining 8 fold inside the
                # transpose-accumulate matmuls (x identity) on PE.
                nc.vector.tensor_tensor(out=pr[:, :8 * SL], in0=pr[:, :8 * SL],
                                        in1=pr[:, 8 * SL:], op=OP.add)
                rp = ps_r.tile([128, 2, 128], F32, space="PSUM", tag="rp")
                for sh in range(2):
                    for hb in range(8):
                        nc.tensor.matmul(
                            rp[:, sh, :],
                            pr[:, hb * SL + sh * 128: hb * SL + sh * 128 + 128],
                            ident[:, :], start=(hb == 0), stop=(hb == 7))
                    nc.scalar.copy(out=rawT[:, sh, pt, :], in_=rp[:, sh, :])
                c, odd = pt // 2, pt % 2
                nc.sync.dma_start(
                    out=a2a_in[odd][c].rearrange("(sh sp) p -> sp sh p", sh=2),
                    in_=rawT[:, :, pt, :])

            with tc.tile_pool(name="pg", bufs=2) as pg, \
                 tc.tile_pool(name="pg1", bufs=1) as pg1, \
                 tc.tile_pool(name="prod", bufs=2) as prod, \
                 tc.tile_pool(name="ps_r", bufs=3, space="PSUM") as ps_r, \
                 tc.tile_pool(name="psg", bufs=2, space="PSUM") as psg:
                for c in range(NC):
                    pair_tile(2 * c, True, pg, pg1, prod, ps_r, psg)
                # a2a #A fires as soon as the evens are staged; odds still run.
                nc.gpsimd.collective_compute(
                    "AllToAll", OP.bypass, replica_groups=[list(range(NC))],
                    ins=[a2a_in[0][:, :, :]], outs=[a2a_out[0][:, :, :]])
                nc.sync.dma_start(
                    out=paT[:, :, 0:128],
                    in_=a2a_out[0].rearrange("j (sh sp) q -> sp (j sh) q", sh=2))

                # EW = ent_emb @ W on PE while a2a #A + the odd gathers run
                with tc.tile_pool(name="ps_e", bufs=1, space="PSUM") as ps_e:
                    for (wsb, dstw) in ((whb, EWh), (wtb, EWt)):
                        for et in range(2):
                            ep = ps_e.tile([128, PH], F32, space="PSUM", tag="ew")
                            for kt in range(H // 128):
                                for nch in range(2):
                                    nc.tensor.matmul(
                                        ep[:, nch * 512:(nch + 1) * 512],
                                        entTe[:, kt, et * 128:(et + 1) * 128],
                                        wsb[:, kt, nch * 512:(nch + 1) * 512],
                                        start=(kt == 0), stop=(kt == H // 128 - 1))
                            nc.scalar.copy(out=dstw[:, et, :], in_=ep[:, :])

                for c in range(NC):
                    pair_tile(2 * c + 1, False, pg, pg1, prod, ps_r, psg)
                nc.gpsimd.collective_compute(
                    "AllToAll", OP.bypass, replica_groups=[list(range(NC))],
                    ins=[a2a_in[1][:, :, :]], outs=[a2a_out[1][:, :, :]])
                nc.sync.dma_start(
                    out=paT[:, :, 128:256],
                    in_=a2a_out[1].rearrange("j (sh sp) q -> sp (j sh) q", sh=2))
                if debug:
                    nc.sync.dma_start(
                        out=dbg["rawT"][:, :],
                        in_=rawT.rearrange("p a b c -> p (a b c)"))

            # ---------------- phases 4..6: pair-half tails ----------------
            with tc.tile_pool(name="ps_tail", bufs=2, space="PSUM") as ps_tail, \
                 tc.tile_pool(name="zscr", bufs=2) as zscr:
                ps_u = ps_z = ps_b = ps_tail
                for hf in range(2):
                    q0, q1 = hf * 128, hf * 128 + 128

                    # ---- contexts (unnormalized) + Z ----
                    zp = ps_u.tile([1, 128], F32, space="PSUM", tag="zr")
                    for t in range(S // 128):
                        nc.tensor.matmul(
                            zp[:, :], ones_col[:, :], paT[:, t, q0:q1],
                            start=(t == 0), stop=(t == S // 128 - 1))
                    nc.scalar.copy(out=zrow[:, q0:q1], in_=zp[:, :])
                    for mc in range(H // 128):
                        up = ps_u.tile([128, 128], F32, space="PSUM", tag="work")
                        for t in range(S // 128):
                            nc.tensor.matmul(
                                up[:, :], seqx[:, t, mc * 128:(mc + 1) * 128],
                                paT[:, t, q0:q1], start=(t == 0),
                                stop=(t == S // 128 - 1))
                        nc.scalar.copy(out=ucb[:, mc, q0:q1], in_=up[:, :])

                    # recip(Z + 1e-6) -> broadcast to 128 partitions via K=1 matmul
                    nc.vector.tensor_scalar_add(out=zrow[:, q0:q1], in0=zrow[:, q0:q1],
                                                scalar1=1e-6)
                    nc.vector.reciprocal(out=zrow[:, q0:q1], in_=zrow[:, q0:q1])
                    nc.vector.tensor_copy(out=zrec_b[:, q0:q1], in_=zrow[:, q0:q1])
                    zrp = ps_u.tile([128, 128], F32, space="PSUM", tag="work")
                    nc.tensor.matmul(zrp[:, :], ones_row[:, :], zrec_b[:, q0:q1],
                                     start=True, stop=True)
                    nc.scalar.copy(out=zrec[:, q0:q1], in_=zrp[:, :])
                    for mc in range(H // 128):
                        nc.vector.tensor_tensor(out=ctxT[:, mc, :],
                                                in0=ucb[:, mc, q0:q1],
                                                in1=zrec[:, q0:q1], op=OP.mult)

                    # ---- z_s, z_o (ctx matmul shared) ----
                    for jt in range(PH // 128):
                        cps = ps_z.tile([128, 128], F32, space="PSUM", tag="work")
                        for kt in range(H // 128):
                            nc.tensor.matmul(
                                cps[:, :], wcb[:, kt, jt * 128:(jt + 1) * 128],
                                ctxT[:, kt, :], start=(kt == 0),
                                stop=(kt == H // 128 - 1))
                        cpsb = zscr.tile([128, 128], BF16, tag="cpsb")
                        nc.scalar.copy(out=cpsb[:, :], in_=cps[:, :])
                        for (ew, oh, bias, dstz, tg) in ((EWh, ohh_t, bhs_t, zsT, "zs"),
                                                         (EWt, oht_t, bts_t, zoT, "zo")):
                            zps = ps_z.tile([128, 128], F32, space="PSUM", tag="zps")
                            for et in range(2):
                                nc.tensor.matmul(
                                    zps[:, :], ew[:, et, jt * 128:(jt + 1) * 128],
                                    oh[:, et, q0:q1], start=(et == 0), stop=(et == 1))
                            nc.vector.tensor_tensor(out=zps[:, :], in0=zps[:, :],
                                                    in1=cpsb[:, :], op=OP.add)
                            nc.scalar.activation(out=dstz[:, jt, :], in_=zps[:, :],
                                                 func=AF.Tanh, bias=bias[:, jt:jt + 1])
                        if debug:
                            nc.vector.tensor_copy(out=dbg_zs[:, jt, q0:q1],
                                                  in_=zsT[:, jt, :])

                    # ---- bilinear ----
                    lg = ps_b.tile([1, 128], F32, space="PSUM", tag="lg")
                    for jt in range(PH // 128):
                        ups = ps_b.tile([128, 128], F32, space="PSUM", tag="work")
                        for it in range(PH // 128):
                            nc.tensor.matmul(
                                ups[:, :], wbb[:, it, jt * 128:(jt + 1) * 128],
                                zsT[:, it, :], start=(it == 0),
                                stop=(it == PH // 128 - 1))
                        pb = zscr.tile([128, 128], BF16, tag="pb")
                        nc.vector.tensor_tensor(out=pb[:, :], in0=ups[:, :],
                                                in1=zoT[:, jt, :], op=OP.mult)
                        nc.tensor.matmul(
                            lg[:, :], ones_col[:, :], pb[:, :],
                            start=(jt == 0), stop=(jt == PH // 128 - 1))
                    nc.vector.tensor_scalar_add(out=lg_sb[:, q0:q1], in0=lg[:, :],
                                                scalar1=bbil_t[:, 0:1])
                if debug:
                    nc.sync.dma_start(
                        out=dbg["ctxuT"].rearrange("(t p) q -> p t q", p=128), in_=ucb)
                    nc.sync.dma_start(out=dbg["zrow"][:, :], in_=zrow)
                    nc.sync.dma_start(
                        out=dbg["zsT"].rearrange("(t p) q -> p t q", p=128), in_=dbg_zs)
                nc.sync.dma_start(out=out[:, :], in_=lg_sb)

    nc.finalize()
    return nc


def _get_nc(mt_ets, debug=False):
    key = ("nc", mt_ets, debug, tuple(sorted(FLAGS.items())))
    if key not in _CACHE:
        _CACHE[key] = _build(mt_ets, debug)
    return _CACHE[key]


def _prep_in_maps(inputs):
    import ml_dtypes
    bf16 = ml_dtypes.bfloat16
    f8 = ml_dtypes.float8_e4m3

    att = np.asarray(inputs["attention"], np.float32)          # [16, 2048, 2048]
    seq = np.asarray(inputs["sequence_output"], np.float32)
    mention_idx = np.asarray(inputs["mention_idx"], np.int32)  # [1024]
    entity_ids = np.asarray(inputs["entity_ids"], np.int32)    # [1024]
    pair_h = np.asarray(inputs["pair_h"], np.int32)            # [2048]
    pair_t = np.asarray(inputs["pair_t"], np.int32)

    counts = np.bincount(entity_ids, minlength=E).astype(np.float32)
    inv_cnt = 1.0 / np.maximum(counts, 1.0)

    ohe = np.zeros((NM, E), np.float32)
    ohe[np.arange(NM), entity_ids] = 1.0
    ohm = np.zeros((NM, E), np.float32)
    ohm[np.arange(NM), entity_ids] = inv_cnt[entity_ids]
    has0r = (counts == 0).astype(np.float32)[None, :]

    # which entity-128-halves each mention tile touches (all-zero slabs skipped)
    mt_ets = tuple(
        tuple(sorted(set((entity_ids[mt * 128:(mt + 1) * 128] // 128).tolist())))
        for mt in range(NMT))

    m_off = mention_idx.reshape(NMT, 128).T.copy()             # [128, 8]

    order = np.argsort(pair_h, kind="stable")
    sph = pair_h[order]
    spt = pair_t[order]
    p_off = np.zeros((128, 2 * NPT), np.int32)
    for pt in range(NPT):
        seg = slice(pt * 128, (pt + 1) * 128)
        p_off[:, 2 * pt] = sph[seg]
        p_off[:, 2 * pt + 1] = spt[seg]

    att8 = att.astype(f8)                                      # [16, 2048, 2048]

    # one-hots for the PE head-gathers of the even pair tiles
    ohg_np = np.zeros((E, 8, 128), np.float32)
    for i in range(8):
        ohg_np[sph[(2 * i) * 128:(2 * i + 1) * 128], i, np.arange(128)] = 1.0

    shared = {
        "seqb": seq.astype(bf16),
        "m_off": m_off,
        "p_off": p_off,
        "ohe": ohe.astype(bf16),
        "ohm": ohm.astype(f8),
        "ohg": ohg_np.reshape(E, 8 * 128).astype(f8),
        "has0r": has0r,
        "w_head": np.asarray(inputs["W_head"], np.float32).astype(bf16),
        "w_tail": np.asarray(inputs["W_tail"], np.float32).astype(bf16),
        "w_ctx": np.asarray(inputs["W_ctx"], np.float32).astype(bf16),
        "w_bil": np.asarray(inputs["W_bil"], np.float32).astype(bf16),
        "b_head": np.asarray(inputs["b_head"], np.float32).reshape(PH // 128, 128).T.copy(),
        "b_tail": np.asarray(inputs["b_tail"], np.float32).reshape(PH // 128, 128).T.copy(),
        "b_bil": np.asarray(inputs["b_bil"], np.float32).reshape(1, 1),
    }

    in_maps = []
    for k in range(NC):
        sk = k * SL
        att_kk = np.ascontiguousarray(
            att8[:, :, sk:sk + SL].transpose(1, 0, 2)).reshape(S, HS)
        ohh_kk = np.zeros((E, PL), np.float32)
        ohh_kk[sph[k * PL:(k + 1) * PL], np.arange(PL)] = 1.0
        oht_kk = np.zeros((E, PL), np.float32)
        oht_kk[spt[k * PL:(k + 1) * PL], np.arange(PL)] = 1.0
        m = dict(shared)
        m["att_k"] = att_kk
        m["ohh_k"] = ohh_kk.astype(bf16)
        m["oht_k"] = oht_kk.astype(bf16)
        in_maps.append(m)
    return in_maps, mt_ets


def _run(inputs, trace=False, debug=False):
    _ensure_axon_profile_hook()
    from concourse.bass_utils import run_bass_kernel_spmd
    in_maps, mt_ets = _prep_in_maps(inputs)
    nc = _get_nc(mt_ets, debug)
    res = run_bass_kernel_spmd(nc, in_maps, list(range(NC)), trace=trace)
    sorted_logits = np.concatenate([np.asarray(res.results[k]["out"][0], np.float32)
                                    for k in range(NC)])
    order = np.argsort(np.asarray(inputs["pair_h"], np.int32), kind="stable")
    logits = np.empty(P, np.float32)
    logits[order] = sorted_logits
    return logits, res


def kernel(**inputs) -> np.ndarray:
    logits, _ = _run(inputs, trace=False)
    return logits


def kernel_traced(**inputs):
    logits, res = _run(inputs, trace=True)
    return logits, res


def kernel_debug(**inputs):
    logits, res = _run(inputs, trace=False, debug=True)
    return logits, res


# revision 15
# speedup vs baseline: 1.1750x; 1.1750x over previous
"""Trainium2 Bass kernel for nn_CandidateFilterModel (segment_reduce).

Strategy (8 cores, S-column sharding for the heavy phases, pair sharding for the tail):
  - Core k owns sequence-column slice s_k = [256k, 256k+256).
  - Phase 1: entity aggregation.
      ent_emb^T = log(OH_emb-matmul of exp(seq[mention_idx]))   (replicated, bf16)
      ent_att (local s-slice) = OH_mean-matmul of gathered attention rows (fp8)
      One-hot slabs that are all-zero (entity_ids is sorted, so each mention
      tile only spans ~32 entities) are skipped entirely.
  - Phase 2: pair products. For all 2048 pairs: gather ent_att rows of head/tail
      entity (4KB fp8 rows, indirect DMA), multiply (fp8 in, bf16 out), one DVE
      add folds 16 heads -> 8, then PE transpose-ACCUMULATE matmuls (x identity)
      fold the remaining 8 head-blocks while transposing -> raw^T in PSUM.
  - Phase 3: TWO AllToAlls (even pair-tiles = first 128 pairs of each dest
      core, then odd) redistribute raw^T so core k holds raw^T[:, P_k].
  - Phases 4-6 (per pair-half): contexts via seq^T-matmul, normalize, z_s/z_o
      via (ent_emb @ W)-then-gather one-hot matmuls + W_ctx matmuls + tanh,
      bilinear via W_bil matmuls + elementwise + ones-reduction matmul.
Host pre-casts: attention fp8 e4m3 (quantization error largely cancels in the
pair_att normalization), seq/weights bf16. DMA queues: gpsimd = indirect
gathers + collectives, sync = small loads/staging/paT, scalar = weight loads.
PSUM->SBUF copies in the tail ride the scalar engine to keep DVE free.
"""
import sys
import types
import numpy as np

S, H, HEADS = 2048, 1024, 16
E, NM, P = 256, 1024, 2048
PH = 1024
NC = 8
SL = S // NC          # 256 s-columns per core
PL = P // NC          # 256 pairs per core
NMT = NM // 128       # 8 mention tiles
NPT = P // 128        # 16 pair tiles
HS = HEADS * SL       # 4096 = width of per-core ent_att rows

_CACHE = {}

# feature flags (bisectable); read at build time and folded into the cache key
FLAGS = {
    "PH1_MERGED": False,   # 2-col merged gathers: CRASHES HW (worker hangup)
    "SCALAR_F8": True,     # entA PSUM->fp8 copies on scalar engine
    "PE_GATHER": True,     # PE one-hot head-gather for even pair tiles
    "CAST_GATHER": True,   # fp8->bf16 cast during th/tt gathers
}


def _ensure_axon_profile_hook():
    """bass_utils' trace path imports antenv.axon_hooks, absent in this image."""
    if 'antenv.axon_hooks' in sys.modules:
        return
    try:
        import antenv.axon_hooks  # noqa: F401
        return
    except ImportError:
        pass
    mod = types.ModuleType('antenv.axon_hooks')
    holder = [None]
    mod.set_axon_ntff_profile_hook = lambda h: holder.__setitem__(0, h)
    mod.get_axon_ntff_profile_hook = lambda: holder[0]
    sys.modules['antenv.axon_hooks'] = mod
    try:
        from trn_agent_boot.trn_boot import _ntff_profile_via_ctypes
        hook = _ntff_profile_via_ctypes('/opt/axon/libaxon_pjrt.so')
        if hook is not None:
            mod.set_axon_ntff_profile_hook(hook)
    except Exception:
        pass


def _build(mt_ets, debug=False):
    """mt_ets: per mention-tile, tuple of entity-128-halves it touches."""
    import concourse.bass as bass
    import concourse.bacc as bacc
    import concourse.tile as tile
    from concourse import mybir
    from concourse.masks import make_identity

    F32 = mybir.dt.float32
    BF16 = mybir.dt.bfloat16
    F8 = mybir.dt.float8e4
    I32 = mybir.dt.int32
    AF = mybir.ActivationFunctionType
    OP = mybir.AluOpType

    nc = bacc.Bacc(num_devices=NC)

    # ---------------- inputs ----------------
    att_k = nc.declare_dram_parameter("att_k", [S, HS], F8, isOutput=False)
    seqb = nc.declare_dram_parameter("seqb", [S, H], BF16, isOutput=False)
    m_off = nc.declare_dram_parameter("m_off", [128, NMT], I32, isOutput=False)
    p_off = nc.declare_dram_parameter("p_off", [128, 2 * NPT], I32, isOutput=False)
    cnts = nc.declare_dram_parameter("cnts", [S, E], BF16, isOutput=False)
    ohm = nc.declare_dram_parameter("ohm", [NM, E], F8, isOutput=False)
    has0r = nc.declare_dram_parameter("has0r", [1, E], F32, isOutput=False)
    ohg = nc.declare_dram_parameter("ohg", [E, NPT * 128], F8, isOutput=False)
    ohh_k = nc.declare_dram_parameter("ohh_k", [E, PL], BF16, isOutput=False)
    oht_k = nc.declare_dram_parameter("oht_k", [E, PL], BF16, isOutput=False)
    w_head = nc.declare_dram_parameter("w_head", [H, PH], BF16, isOutput=False)
    w_tail = nc.declare_dram_parameter("w_tail", [H, PH], BF16, isOutput=False)
    w_ctx = nc.declare_dram_parameter("w_ctx", [H, PH], BF16, isOutput=False)
    w_bil = nc.declare_dram_parameter("w_bil", [PH, PH], BF16, isOutput=False)
    b_head = nc.declare_dram_parameter("b_head", [128, PH // 128], F32, isOutput=False)
    b_tail = nc.declare_dram_parameter("b_tail", [128, PH // 128], F32, isOutput=False)
    b_bil = nc.declare_dram_parameter("b_bil", [1, 1], F32, isOutput=False)
    out = nc.declare_dram_parameter("out", [1, PL], F32, isOutput=True)

    dbg = {}
    if debug:
        dbg["ent_embT"] = nc.declare_dram_parameter("d_ent_embT", [H, E], BF16, isOutput=True)
        dbg["entA"] = nc.declare_dram_parameter("d_entA", [E, HS], BF16, isOutput=True)
        dbg["rawT"] = nc.declare_dram_parameter("d_rawT", [128, 2 * NPT * 128], BF16, isOutput=True)
        dbg["ctxnT"] = nc.declare_dram_parameter("d_ctxnT", [H, PL], BF16, isOutput=True)
        dbg["zrec"] = nc.declare_dram_parameter("d_zrec", [128, 2], F32, isOutput=True)
        dbg["zsT"] = nc.declare_dram_parameter("d_zsT", [PH, PL], BF16, isOutput=True)

    # internal DRAM
    entA_dram = nc.dram_tensor("entA_dram", [E, HS], F8)
    a2a_in = [nc.dram_tensor(f"a2a{h}_in", [NC, SL, 128], BF16) for h in range(2)]
    a2a_out = [nc.dram_tensor(f"a2a{h}_out", [NC, SL, 128], BF16) for h in range(2)]

    et_mts = {0: [mt for mt in range(NMT) if 0 in mt_ets[mt]],
              1: [mt for mt in range(NMT) if 1 in mt_ets[mt]]}

    with tile.TileContext(nc) as tc:
        with tc.tile_pool(name="singles", bufs=1) as singles, \
             tc.tile_pool(name="wpool", bufs=1) as wpool:
            # ---------------- phase 0: small loads (sync queue) ----------------
            m_off_t = singles.tile([128, NMT], I32)
            nc.sync.dma_start(out=m_off_t, in_=m_off[:, :])
            p_off_t = singles.tile([128, 2 * NPT], I32)
            nc.sync.dma_start(out=p_off_t, in_=p_off[:, :])
            ohg_t = singles.tile([128, 2, NPT, 128], F8)
            nc.sync.dma_start(out=ohg_t, in_=ohg.rearrange("(t p) (i q) -> p t i q", p=128, q=128))
            ohh_t = singles.tile([128, 2, PL], BF16)
            nc.sync.dma_start(out=ohh_t, in_=ohh_k.rearrange("(t p) q -> p t q", p=128))
            oht_t = singles.tile([128, 2, PL], BF16)
            nc.sync.dma_start(out=oht_t, in_=oht_k.rearrange("(t p) q -> p t q", p=128))
            bhs_t = singles.tile([128, PH // 128], F32)
            nc.sync.dma_start(out=bhs_t, in_=b_head[:, :])
            bts_t = singles.tile([128, PH // 128], F32)
            nc.sync.dma_start(out=bts_t, in_=b_tail[:, :])
            bbil_t = singles.tile([1, 1], F32)
            nc.sync.dma_start(out=bbil_t, in_=b_bil[:, :])
            ident = singles.tile([128, 128], BF16)
            make_identity(nc, ident[:, :])
            # warm activation tables; Exp last = first real user
            warm = singles.tile([1, 8], F32)
            nc.vector.memset(warm[:, :], 0.0)
            nc.scalar.activation(out=warm[:, :], in_=warm[:, :], func=AF.Tanh)
            nc.scalar.activation(out=warm[:, :], in_=warm[:, :], func=AF.Ln)
            nc.scalar.activation(out=warm[:, :], in_=warm[:, :], func=AF.Exp)
            ones_col = singles.tile([128, 1], BF16)
            nc.vector.memset(ones_col[:, :], 1.0)

            entTe = singles.tile([128, H // 128, E], BF16)  # ent_emb^T [hcol-part, hc, e]
            entA_sb = singles.tile([128, 2, HS], F8)        # ent_att fp8 [e-part, et, (h s)]
            rawT = singles.tile([128, 2, NPT, 128], BF16)   # [s-part, sh, pt, p-row]
            paT = singles.tile([128, S // 128, PL], BF16)   # raw^T for my pairs, all s
            ctxT = singles.tile([128, H // 128, 128], BF16)
            ctxp_sb = singles.tile([128, H], BF16)          # normalized contexts [p, h]
            zsT = singles.tile([128, PH // 128, 128], BF16)
            zoT = singles.tile([128, PH // 128, 128], BF16)
            EWh = singles.tile([128, 2, PH], BF16)          # ent_emb @ W_head [e-part, et, PH]
            EWt = singles.tile([128, 2, PH], BF16)
            lg_sb = singles.tile([1, PL], F32)
            dbg_zs = singles.tile([128, PH // 128, PL], BF16) if debug else None
            dbg_ctx = singles.tile([128, H // 128, PL], BF16) if debug else None
            dbg_zr = singles.tile([128, 2], F32) if debug else None

            # weight tiles; loads are emitted on the scalar HWDGE queue after
            # the lse exps so they don't steal HBM from the att gathers.
            whb = wpool.tile([128, H // 128, PH], BF16)
            wtb = wpool.tile([128, H // 128, PH], BF16)
            wcb = wpool.tile([128, H // 128, PH], BF16)
            wbb = wpool.tile([128, PH // 128, PH], BF16)
            seqx = wpool.tile([128, S // 128, H], BF16)
            # seqx in 4 chunks so the first lse exp starts early
            for c4 in range(4):
                nc.scalar.dma_start(
                    out=seqx[:, 4 * c4:4 * c4 + 4, :],
                    in_=seqb.rearrange("(t p) h -> p t h", p=128)[:, 4 * c4:4 * c4 + 4, :])

            # ---------------- phase 1: att gathers + lse + aggregation ----------
            with tc.tile_pool(name="p1", bufs=1) as p1:
                cnts_t = p1.tile([128, S // 128, E], BF16)
                nc.sync.dma_start(out=cnts_t, in_=cnts.rearrange("(t p) e -> p t e", p=128))
                ohm_t = p1.tile([128, NMT, E], F8)
                nc.sync.dma_start(out=ohm_t, in_=ohm.rearrange("(t p) e -> p t e", p=128))
                has0b = p1.tile([128, E], F32)
                nc.sync.dma_start(out=has0b, in_=has0r[:, :].to_broadcast([128, E]))
                ag = []
                for mt in range(NMT):
                    g = p1.tile([128, HS], F8, tag=f"ag{mt}")
                    nc.gpsimd.indirect_dma_start(
                        out=g[:, :], out_offset=None, in_=att_k[:, :],
                        in_offset=bass.IndirectOffsetOnAxis(ap=m_off_t[:, mt:mt + 1], axis=0))
                    ag.append(g)

                # logsumexp sums via streaming exp(seq) x counts matmul (no
                # mention gather): sums[h, e] = sum_s exp(seq[s, h]) cnts[s, e]
                with tc.tile_pool(name="ps_l", bufs=1, space="PSUM") as ps_l, \
                     tc.tile_pool(name="expp", bufs=2) as expp:
                    sums_ps = []
                    for hc in range(H // 128):
                        sps = ps_l.tile([128, E], F32, space="PSUM", tag=f"sums{hc}")
                        sums_ps.append(sps)
                    for st in range(S // 128):
                        ex = expp.tile([128, H], BF16, tag="ex")
                        nc.scalar.activation(out=ex[:, :], in_=seqx[:, st, :], func=AF.Exp)
                        for hc in range(H // 128):
                            nc.tensor.matmul(
                                sums_ps[hc][:, :], ex[:, hc * 128:(hc + 1) * 128],
                                cnts_t[:, st, :], start=(st == 0),
                                stop=(st == S // 128 - 1))
                    for hc in range(H // 128):
                        nc.vector.tensor_tensor(out=sums_ps[hc][:, :], in0=sums_ps[hc][:, :],
                                                in1=has0b[:, :], op=OP.add)
                        nc.scalar.activation(out=entTe[:, hc, :], in_=sums_ps[hc][:, :],
                                             func=AF.Ln)
                if debug:
                    nc.sync.dma_start(
                        out=dbg["ent_embT"].rearrange("(t p) e -> p t e", p=128), in_=entTe)

                # weight loads fire once the scalar queue reaches them (post-exp)
                nc.scalar.dma_start(out=whb, in_=w_head.rearrange("(t p) n -> p t n", p=128))
                nc.scalar.dma_start(out=wtb, in_=w_tail.rearrange("(t p) n -> p t n", p=128))
                nc.scalar.dma_start(out=wcb, in_=w_ctx.rearrange("(t p) n -> p t n", p=128))
                nc.scalar.dma_start(out=wbb, in_=w_bil.rearrange("(t p) n -> p t n", p=128))

                # entity attention aggregation (one-hot matmuls, windowed)
                with tc.tile_pool(name="ps_a", bufs=1, space="PSUM") as ps_a:
                    for hg in range(2):
                        pas0 = ps_a.tile([128, 8 * SL], F32, space="PSUM", tag="agg0")
                        pas1 = ps_a.tile([128, 8 * SL], F32, space="PSUM", tag="agg1")
                        pas = {0: pas0, 1: pas1}
                        for et in range(2):
                            if not et_mts[et]:
                                nc.vector.memset(pas[et][:, :], 0.0)
                        for mt in range(NMT):
                            for et in mt_ets[mt]:
                                for nch in range(4):  # 2048 = 4 x 512
                                    nc.tensor.matmul(
                                        pas[et][:, nch * 512:(nch + 1) * 512],
                                        ohm_t[:, mt, et * 128:(et + 1) * 128],
                                        ag[mt][:, hg * 2048 + nch * 512:
                                               hg * 2048 + (nch + 1) * 512],
                                        start=(mt == et_mts[et][0]),
                                        stop=(mt == et_mts[et][-1]))
                        for et in range(2):
                            nc.scalar.copy(
                                out=entA_sb[:, et, hg * 2048:(hg + 1) * 2048],
                                in_=pas[et][:, :])
                            nc.sync.dma_start(
                                out=entA_dram.rearrange("(t p) w -> p t w", p=128)[
                                    :, et, hg * 2048:(hg + 1) * 2048],
                                in_=entA_sb[:, et, hg * 2048:(hg + 1) * 2048])
                            if debug:
                                eAb = p1.tile([128, 8 * SL], BF16, tag="entA_dbg")
                                nc.vector.tensor_copy(out=eAb[:, :], in_=pas[et][:, :])
                                nc.sync.dma_start(
                                    out=dbg["entA"].rearrange("(t p) w -> p t w", p=128)[
                                        :, et, hg * 2048:(hg + 1) * 2048],
                                    in_=eAb[:, :])

            # ---------------- phase 2: pair products ----------------
            # evens (tiles 0,2,..,14 = first 128 pairs of each dest core) first
            # so AllToAll #A can fire while the odds still stream. Head rows
            # come from PE one-hot matmuls out of SBUF for 12/16 tiles (pairs
            # sorted by head; Q7 descgen is the limiting serial resource);
            # tails and 4 odd heads are Q7 gathers casting fp8->bf16.
            def pair_tile(pt, pe_head, pg, pg1, prod, ps_r, psg):
                if pe_head:
                    th = pg.tile([128, HS], BF16, tag="thp")
                    for ch in range(8):
                        gp = psg.tile([128, 512], F32, space="PSUM", tag="gp")
                        for et in range(2):
                            nc.tensor.matmul(
                                gp[:, :], ohg_t[:, et, pt, :],
                                entA_sb[:, et, ch * 512:(ch + 1) * 512],
                                start=(et == 0), stop=(et == 1))
                        nc.scalar.copy(out=th[:, ch * 512:(ch + 1) * 512], in_=gp[:, :])
                else:
                    th = pg1.tile([128, HS], BF16, tag="th")
                    nc.gpsimd.indirect_dma_start(
                        out=th[:, :], out_offset=None, in_=entA_dram[:, :],
                        in_offset=bass.IndirectOffsetOnAxis(
                            ap=p_off_t[:, 2 * pt:2 * pt + 1], axis=0))
                tt = pg.tile([128, HS], BF16, tag="tt")
                nc.gpsimd.indirect_dma_start(
                    out=tt[:, :], out_offset=None, in_=entA_dram[:, :],
                    in_offset=bass.IndirectOffsetOnAxis(
                        ap=p_off_t[:, 2 * pt + 1:2 * pt + 2], axis=0))
                pr = prod.tile([128, HS], BF16, tag="pr")
                nc.vector.tensor_tensor(out=pr[:, :], in0=th[:, :], in1=tt[:, :],
                                        op=OP.mult)
                # fold 16 heads -> 8 on DVE; remaining 8 fold inside the
                # transpose-accumulate matmuls (x identity) on PE.
                nc.vector.tensor_tensor(out=pr[:, :8 * SL], in0=pr[:, :8 * SL],
                                        in1=pr[:, 8 * SL:], op=OP.add)
                rp = ps_r.tile([128, 2, 128], F32, space="PSUM", tag="rp")
                for sh in range(2):
                    for hb in range(8):
                        nc.tensor.matmul(
                            rp[:, sh, :],
                            pr[:, hb * SL + sh * 128: hb * SL + sh * 128 + 128],
                            ident[:, :], start=(hb == 0), stop=(hb == 7))
                    nc.scalar.copy(out=rawT[:, sh, pt, :], in_=rp[:, sh, :])
                c, odd = pt // 2, pt % 2
                nc.sync.dma_start(
                    out=a2a_in[odd][c].rearrange("(sh sp) p -> sp sh p", sh=2),
                    in_=rawT[:, :, pt, :])

            with tc.tile_pool(name="pg", bufs=2) as pg, \
                 tc.tile_pool(name="pg1", bufs=1) as pg1, \
                 tc.tile_pool(name="prod", bufs=2) as prod, \
                 tc.tile_pool(name="ps_r", bufs=3, space="PSUM") as ps_r, \
                 tc.tile_pool(name="psg", bufs=2, space="PSUM") as psg, \
                 tc.tile_pool(name="ps_e", bufs=1, space="PSUM") as ps_e:
                for c in range(NC):
                    pair_tile(2 * c, True, pg, pg1, prod, ps_r, psg)
                # a2a #A fires as soon as the evens are staged; odds still run.
                nc.gpsimd.collective_compute(
                    "AllToAll", OP.bypass, replica_groups=[list(range(NC))],
                    ins=[a2a_in[0][:, :, :]], outs=[a2a_out[0][:, :, :]])
                nc.sync.dma_start(
                    out=paT[:, :, 0:128],
                    in_=a2a_out[0].rearrange("j (sh sp) q -> sp (j sh) q", sh=2))
                for c in range(NC):
                    pair_tile(2 * c + 1, c % 2 == 0, pg, pg1, prod, ps_r, psg)
                nc.gpsimd.collective_compute(
                    "AllToAll", OP.bypass, replica_groups=[list(range(NC))],
                    ins=[a2a_in[1][:, :, :]], outs=[a2a_out[1][:, :, :]])
                nc.sync.dma_start(
                    out=paT[:, :, 128:256],
                    in_=a2a_out[1].rearrange("j (sh sp) q -> sp (j sh) q", sh=2))
                if debug:
                    nc.sync.dma_start(
                        out=dbg["rawT"][:, :],
                        in_=rawT.rearrange("p a b c -> p (a b c)"))

                # EW = ent_emb @ W on PE while a2a #A completes
                for (wsb, dstw) in ((whb, EWh), (wtb, EWt)):
                    for et in range(2):
                        ep = ps_e.tile([128, PH], F32, space="PSUM", tag="ew")
                        for kt in range(H // 128):
                            for nch in range(2):
                                nc.tensor.matmul(
                                    ep[:, nch * 512:(nch + 1) * 512],
                                    entTe[:, kt, et * 128:(et + 1) * 128],
                                    wsb[:, kt, nch * 512:(nch + 1) * 512],
                                    start=(kt == 0), stop=(kt == H // 128 - 1))
                        nc.scalar.copy(out=dstw[:, et, :], in_=ep[:, :])

            # ---------------- phases 4..6: pair-half tails ----------------
            with tc.tile_pool(name="psA", bufs=1, space="PSUM") as psA, \
                 tc.tile_pool(name="psB", bufs=2, space="PSUM") as psB, \
                 tc.tile_pool(name="zscr", bufs=2) as zscr:
                for hf in range(2):
                    q0, q1 = hf * 128, hf * 128 + 128

                    # Z (pairs on partitions) then unnormalized contexts [p, h]
                    zp2 = psA.tile([128, 1], F32, space="PSUM", tag="z2")
                    for t in range(S // 128):
                        nc.tensor.matmul(
                            zp2[:, :], paT[:, t, q0:q1], ones_col[:, :],
                            start=(t == 0), stop=(t == S // 128 - 1))
                    zr = zscr.tile([128, 1], F32, tag="zr")
                    nc.vector.tensor_scalar_add(out=zr[:, :], in0=zp2[:, :],
                                                scalar1=1e-6)
                    nc.vector.reciprocal(out=zr[:, :], in_=zr[:, :])
                    if debug:
                        nc.vector.tensor_copy(out=dbg_zr[:, hf:hf + 1], in_=zr[:, :])
                    ucp = psA.tile([128, H], F32, space="PSUM", tag="ucp")
                    for t in range(S // 128):
                        for nchu in range(2):
                            nc.tensor.matmul(
                                ucp[:, nchu * 512:(nchu + 1) * 512],
                                paT[:, t, q0:q1],
                                seqx[:, t, nchu * 512:(nchu + 1) * 512],
                                start=(t == 0), stop=(t == S // 128 - 1))
                    # normalize on scalar (per-partition scale), transpose back
                    nc.scalar.activation(out=ctxp_sb[:, :], in_=ucp[:, :],
                                         func=AF.Copy, scale=zr[:, :])
                    for mc in range(H // 128):
                        tw = psB.tile([128, 128], F32, space="PSUM", tag="work")
                        nc.tensor.matmul(tw[:, :], ctxp_sb[:, mc * 128:(mc + 1) * 128],
                                         ident[:, :], start=True, stop=True)
                        nc.scalar.copy(out=ctxT[:, mc, :], in_=tw[:, :])
                        if debug:
                            nc.vector.tensor_copy(out=dbg_ctx[:, mc, q0:q1],
                                                  in_=ctxT[:, mc, :])

                    # ---- z_s, z_o (ctx matmul shared) ----
                    for jt in range(PH // 128):
                        cps = psB.tile([128, 128], F32, space="PSUM", tag="work")
                        for kt in range(H // 128):
                            nc.tensor.matmul(
                                cps[:, :], wcb[:, kt, jt * 128:(jt + 1) * 128],
                                ctxT[:, kt, :], start=(kt == 0),
                                stop=(kt == H // 128 - 1))
                        cpsb = zscr.tile([128, 128], BF16, tag="cpsb")
                        nc.scalar.copy(out=cpsb[:, :], in_=cps[:, :])
                        for (ew, oh, bias, dstz) in ((EWh, ohh_t, bhs_t, zsT),
                                                     (EWt, oht_t, bts_t, zoT)):
                            zps = psB.tile([128, 128], F32, space="PSUM", tag="zps")
                            for et in range(2):
                                nc.tensor.matmul(
                                    zps[:, :], ew[:, et, jt * 128:(jt + 1) * 128],
                                    oh[:, et, q0:q1], start=(et == 0), stop=(et == 1))
                            nc.vector.tensor_tensor(out=zps[:, :], in0=zps[:, :],
                                                    in1=cpsb[:, :], op=OP.add)
                            nc.scalar.activation(out=dstz[:, jt, :], in_=zps[:, :],
                                                 func=AF.Tanh, bias=bias[:, jt:jt + 1])
                        if debug:
                            nc.vector.tensor_copy(out=dbg_zs[:, jt, q0:q1],
                                                  in_=zsT[:, jt, :])

                    # ---- bilinear ----
                    lg = psA.tile([1, 128], F32, space="PSUM", tag="lg")
                    for jt in range(PH // 128):
                        ups = psB.tile([128, 128], F32, space="PSUM", tag="work")
                        for it in range(PH // 128):
                            nc.tensor.matmul(
                                ups[:, :], wbb[:, it, jt * 128:(jt + 1) * 128],
                                zsT[:, it, :], start=(it == 0),
                                stop=(it == PH // 128 - 1))
                        pb = zscr.tile([128, 128], BF16, tag="pb")
                        nc.vector.tensor_tensor(out=pb[:, :], in0=ups[:, :],
                                                in1=zoT[:, jt, :], op=OP.mult)
                        nc.tensor.matmul(
                            lg[:, :], ones_col[:, :], pb[:, :],
                            start=(jt == 0), stop=(jt == PH // 128 - 1))
                    nc.vector.tensor_scalar_add(out=lg_sb[:, q0:q1], in0=lg[:, :],
                                                scalar1=bbil_t[:, 0:1])
                if debug:
                    nc.sync.dma_start(
                        out=dbg["ctxnT"].rearrange("(t p) q -> p t q", p=128),
                        in_=dbg_ctx)
                    nc.sync.dma_start(out=dbg["zrec"][:, :], in_=dbg_zr)
                    nc.sync.dma_start(
                        out=dbg["zsT"].rearrange("(t p) q -> p t q", p=128), in_=dbg_zs)
                nc.sync.dma_start(out=out[:, :], in_=lg_sb)

    nc.finalize()
    return nc


def _get_nc(mt_ets, debug=False):
    key = ("nc", mt_ets, debug)
    if key not in _CACHE:
        _CACHE[key] = _build(mt_ets, debug)
    return _CACHE[key]


def _prep_in_maps(inputs):
    import ml_dtypes
    bf16 = ml_dtypes.bfloat16
    f8 = ml_dtypes.float8_e4m3

    att = np.asarray(inputs["attention"], np.float32)          # [16, 2048, 2048]
    seq = np.asarray(inputs["sequence_output"], np.float32)
    mention_idx = np.asarray(inputs["mention_idx"], np.int32)  # [1024]
    entity_ids = np.asarray(inputs["entity_ids"], np.int32)    # [1024]
    pair_h = np.asarray(inputs["pair_h"], np.int32)            # [2048]
    pair_t = np.asarray(inputs["pair_t"], np.int32)

    counts = np.bincount(entity_ids, minlength=E).astype(np.float32)
    inv_cnt = 1.0 / np.maximum(counts, 1.0)

    cnts_np = np.zeros((S, E), np.float32)
    np.add.at(cnts_np, (mention_idx, entity_ids), 1.0)
    ohm = np.zeros((NM, E), np.float32)
    ohm[np.arange(NM), entity_ids] = inv_cnt[entity_ids]
    has0r = (counts == 0).astype(np.float32)[None, :]

    # which entity-128-halves each mention tile touches (all-zero slabs skipped)
    mt_ets = tuple(
        tuple(sorted(set((entity_ids[mt * 128:(mt + 1) * 128] // 128).tolist())))
        for mt in range(NMT))

    m_off = mention_idx.reshape(NMT, 128).T.copy()             # [128, 8]

    order = np.argsort(pair_h, kind="stable")
    sph = pair_h[order]
    spt = pair_t[order]
    p_off = np.zeros((128, 2 * NPT), np.int32)
    for pt in range(NPT):
        seg = slice(pt * 128, (pt + 1) * 128)
        p_off[:, 2 * pt] = sph[seg]
        p_off[:, 2 * pt + 1] = spt[seg]

    att8 = att.astype(f8)                                      # [16, 2048, 2048]

    # one-hots for the PE head-gathers (all 16 pair tiles)
    ohg_np = np.zeros((E, NPT, 128), np.float32)
    for i in range(NPT):
        ohg_np[sph[i * 128:(i + 1) * 128], i, np.arange(128)] = 1.0

    shared = {
        "seqb": seq.astype(bf16),
        "m_off": m_off,
        "p_off": p_off,
        "cnts": cnts_np.astype(bf16),
        "ohm": ohm.astype(f8),
        "ohg": ohg_np.reshape(E, NPT * 128).astype(f8),
        "has0r": has0r,
        "w_head": np.asarray(inputs["W_head"], np.float32).astype(bf16),
        "w_tail": np.asarray(inputs["W_tail"], np.float32).astype(bf16),
        "w_ctx": np.asarray(inputs["W_ctx"], np.float32).astype(bf16),
        "w_bil": np.asarray(inputs["W_bil"], np.float32).astype(bf16),
        "b_head": np.asarray(inputs["b_head"], np.float32).reshape(PH // 128, 128).T.copy(),
        "b_tail": np.asarray(inputs["b_tail"], np.float32).reshape(PH // 128, 128).T.copy(),
        "b_bil": np.asarray(inputs["b_bil"], np.float32).reshape(1, 1),
    }

    in_maps = []
    for k in range(NC):
        sk = k * SL
        att_kk = np.ascontiguousarray(
            att8[:, :, sk:sk + SL].transpose(1, 0, 2)).reshape(S, HS)
        ohh_kk = np.zeros((E, PL), np.float32)
        ohh_kk[sph[k * PL:(k + 1) * PL], np.arange(PL)] = 1.0
        oht_kk = np.zeros((E, PL), np.float32)
        oht_kk[spt[k * PL:(k + 1) * PL], np.arange(PL)] = 1.0
        m = dict(shared)
        m["att_k"] = att_kk
        m["ohh_k"] = ohh_kk.astype(bf16)
        m["oht_k"] = oht_kk.astype(bf16)
        in_maps.append(m)
    return in_maps, mt_ets


def _run(inputs, trace=False, debug=False):
    _ensure_axon_profile_hook()
    from concourse.bass_utils import run_bass_kernel_spmd
    in_maps, mt_ets = _prep_in_maps(inputs)
    nc = _get_nc(mt_ets, debug)
    res = run_bass_kernel_spmd(nc, in_maps, list(range(NC)), trace=trace)
    sorted_logits = np.concatenate([np.asarray(res.results[k]["out"][0], np.float32)
                                    for k in range(NC)])
    order = np.argsort(np.asarray(inputs["pair_h"], np.int32), kind="stable")
    logits = np.empty(P, np.float32)
    logits[order] = sorted_logits
    return logits, res


def kernel(**inputs) -> np.ndarray:
    logits, _ = _run(inputs, trace=False)
    return logits


def kernel_traced(**inputs):
    logits, res = _run(inputs, trace=True)
    return logits, res


def kernel_debug(**inputs):
    logits, res = _run(inputs, trace=False, debug=True)
    return logits, res


# revision 17
# speedup vs baseline: 1.2039x; 1.0246x over previous
"""Trainium2 Bass kernel for nn_CandidateFilterModel (segment_reduce).

Strategy (8 cores, S-column sharding for the heavy phases, pair sharding for the tail):
  - Core k owns sequence-column slice s_k = [256k, 256k+256).
  - Phase 1: entity aggregation.
      ent_emb^T = log(OH_emb-matmul of exp(seq[mention_idx]))   (replicated, bf16)
      ent_att (local s-slice) = OH_mean-matmul of gathered attention rows (fp8)
      One-hot slabs that are all-zero (entity_ids is sorted, so each mention
      tile only spans ~32 entities) are skipped entirely.
  - Phase 2: pair products. For all 2048 pairs: gather ent_att rows of head/tail
      entity (4KB fp8 rows, indirect DMA), multiply (fp8 in, bf16 out), one DVE
      add folds 16 heads -> 8, then PE transpose-ACCUMULATE matmuls (x identity)
      fold the remaining 8 head-blocks while transposing -> raw^T in PSUM.
  - Phase 3: TWO AllToAlls (even pair-tiles = first 128 pairs of each dest
      core, then odd) redistribute raw^T so core k holds raw^T[:, P_k].
  - Phases 4-6 (per pair-half): contexts via seq^T-matmul, normalize, z_s/z_o
      via (ent_emb @ W)-then-gather one-hot matmuls + W_ctx matmuls + tanh,
      bilinear via W_bil matmuls + elementwise + ones-reduction matmul.
Host pre-casts: attention fp8 e4m3 (quantization error largely cancels in the
pair_att normalization), seq/weights bf16. DMA queues: gpsimd = indirect
gathers + collectives, sync = small loads/staging/paT, scalar = weight loads.
PSUM->SBUF copies in the tail ride the scalar engine to keep DVE free.
"""
import sys
import types
import numpy as np

S, H, HEADS = 2048, 1024, 16
E, NM, P = 256, 1024, 2048
PH = 1024
NC = 8
SL = S // NC          # 256 s-columns per core
PL = P // NC          # 256 pairs per core
NMT = NM // 128       # 8 mention tiles
NPT = P // 128        # 16 pair tiles
HS = HEADS * SL       # 4096 = width of per-core ent_att rows

_CACHE = {}

# feature flags (bisectable); read at build time and folded into the cache key
FLAGS = {
    "PH1_MERGED": False,   # 2-col merged gathers: CRASHES HW (worker hangup)
    "SCALAR_F8": True,     # entA PSUM->fp8 copies on scalar engine
    "PE_GATHER": True,     # PE one-hot head-gather for even pair tiles
    "CAST_GATHER": True,   # fp8->bf16 cast during th/tt gathers
}


def _ensure_axon_profile_hook():
    """bass_utils' trace path imports antenv.axon_hooks, absent in this image."""
    if 'antenv.axon_hooks' in sys.modules:
        return
    try:
        import antenv.axon_hooks  # noqa: F401
        return
    except ImportError:
        pass
    mod = types.ModuleType('antenv.axon_hooks')
    holder = [None]
    mod.set_axon_ntff_profile_hook = lambda h: holder.__setitem__(0, h)
    mod.get_axon_ntff_profile_hook = lambda: holder[0]
    sys.modules['antenv.axon_hooks'] = mod
    try:
        from trn_agent_boot.trn_boot import _ntff_profile_via_ctypes
        hook = _ntff_profile_via_ctypes('/opt/axon/libaxon_pjrt.so')
        if hook is not None:
            mod.set_axon_ntff_profile_hook(hook)
    except Exception:
        pass


def _build(mt_ets, debug=False):
    """mt_ets: per mention-tile, tuple of entity-128-halves it touches."""
    import concourse.bass as bass
    import concourse.bacc as bacc
    import concourse.tile as tile
    from concourse import mybir
    from concourse.masks import make_identity

    F32 = mybir.dt.float32
    BF16 = mybir.dt.bfloat16
    F8 = mybir.dt.float8e4
    I32 = mybir.dt.int32
    AF = mybir.ActivationFunctionType
    OP = mybir.AluOpType

    nc = bacc.Bacc(num_devices=NC)

    # ---------------- inputs ----------------
    att_k = nc.declare_dram_parameter("att_k", [S, HS], F8, isOutput=False)
    seqp = nc.declare_dram_parameter("seqp", [128, (S // 128) * H], BF16, isOutput=False)
    m_off = nc.declare_dram_parameter("m_off", [128, NMT], I32, isOutput=False)
    p_off = nc.declare_dram_parameter("p_off", [128, 2 * NPT], I32, isOutput=False)
    cnts = nc.declare_dram_parameter("cnts", [128, (S // 128) * E], BF16, isOutput=False)
    ohm = nc.declare_dram_parameter("ohm", [128, NMT * E], F8, isOutput=False)
    has0r = nc.declare_dram_parameter("has0r", [1, E], F32, isOutput=False)
    ohh_k = nc.declare_dram_parameter("ohh_k", [128, 2 * PL], BF16, isOutput=False)
    oht_k = nc.declare_dram_parameter("oht_k", [128, 2 * PL], BF16, isOutput=False)
    w_head = nc.declare_dram_parameter("w_head", [128, (H // 128) * PH], BF16, isOutput=False)
    w_tail = nc.declare_dram_parameter("w_tail", [128, (H // 128) * PH], BF16, isOutput=False)
    w_ctx = nc.declare_dram_parameter("w_ctx", [128, (H // 128) * PH], BF16, isOutput=False)
    w_bil = nc.declare_dram_parameter("w_bil", [128, (PH // 128) * PH], BF16, isOutput=False)
    b_head = nc.declare_dram_parameter("b_head", [128, PH // 128], F32, isOutput=False)
    b_tail = nc.declare_dram_parameter("b_tail", [128, PH // 128], F32, isOutput=False)
    b_bil = nc.declare_dram_parameter("b_bil", [1, 1], F32, isOutput=False)
    out = nc.declare_dram_parameter("out", [1, PL], F32, isOutput=True)

    dbg = {}
    if debug:
        dbg["ent_embT"] = nc.declare_dram_parameter("d_ent_embT", [H, E], BF16, isOutput=True)
        dbg["entA"] = nc.declare_dram_parameter("d_entA", [E, HS], BF16, isOutput=True)
        dbg["rawT"] = nc.declare_dram_parameter("d_rawT", [128, 2 * NPT * 128], BF16, isOutput=True)
        dbg["ctxnT"] = nc.declare_dram_parameter("d_ctxnT", [H, PL], BF16, isOutput=True)
        dbg["zrec"] = nc.declare_dram_parameter("d_zrec", [128, 2], F32, isOutput=True)
        dbg["zsT"] = nc.declare_dram_parameter("d_zsT", [PH, PL], BF16, isOutput=True)

    # internal DRAM
    entA_dram = nc.dram_tensor("entA_dram", [E, HS], F8)
    a2a_in = [nc.dram_tensor(f"a2a{h}_in", [NC, SL, 128], BF16) for h in range(2)]
    a2a_out = [nc.dram_tensor(f"a2a{h}_out", [NC, SL, 128], BF16) for h in range(2)]

    et_mts = {0: [mt for mt in range(NMT) if 0 in mt_ets[mt]],
              1: [mt for mt in range(NMT) if 1 in mt_ets[mt]]}

    with tile.TileContext(nc) as tc:
        with tc.tile_pool(name="singles", bufs=1) as singles, \
             tc.tile_pool(name="wpool", bufs=1) as wpool:
            # ---------------- phase 0: small loads (sync queue) ----------------
            m_off_t = singles.tile([128, NMT], I32)
            nc.sync.dma_start(out=m_off_t, in_=m_off[:, :])
            p_off_t = singles.tile([128, 2 * NPT], I32)
            nc.sync.dma_start(out=p_off_t, in_=p_off[:, :])
            ohh_t = singles.tile([128, 2, PL], BF16)
            nc.sync.dma_start(out=ohh_t, in_=ohh_k[:, :])
            oht_t = singles.tile([128, 2, PL], BF16)
            nc.sync.dma_start(out=oht_t, in_=oht_k[:, :])
            bhs_t = singles.tile([128, PH // 128], F32)
            nc.sync.dma_start(out=bhs_t, in_=b_head[:, :])
            bts_t = singles.tile([128, PH // 128], F32)
            nc.sync.dma_start(out=bts_t, in_=b_tail[:, :])
            bbil_t = singles.tile([1, 1], F32)
            nc.sync.dma_start(out=bbil_t, in_=b_bil[:, :])
            ident = singles.tile([128, 128], BF16)
            make_identity(nc, ident[:, :])
            # warm activation tables; Exp last = first real user
            warm = singles.tile([1, 8], F32)
            nc.vector.memset(warm[:, :], 0.0)
            nc.scalar.activation(out=warm[:, :], in_=warm[:, :], func=AF.Tanh)
            nc.scalar.activation(out=warm[:, :], in_=warm[:, :], func=AF.Ln)
            nc.scalar.activation(out=warm[:, :], in_=warm[:, :], func=AF.Exp)
            ones_col = singles.tile([128, 1], BF16)
            nc.vector.memset(ones_col[:, :], 1.0)

            entTe = singles.tile([128, H // 128, E], BF16)  # ent_emb^T [hcol-part, hc, e]
            entA_sb = singles.tile([128, 2, HS], F8)        # ent_att fp8 [e-part, et, (h s)]
            rawT = singles.tile([128, 2, NPT, 128], BF16)   # [s-part, sh, pt, p-row]
            paT = singles.tile([128, S // 128, PL], BF16)   # raw^T for my pairs, all s
            ctxT = singles.tile([128, H // 128, 128], BF16)
            ctxp_sb = singles.tile([128, H], BF16)          # normalized contexts [p, h]
            zsT = singles.tile([128, PH // 128, 128], BF16)
            zoT = singles.tile([128, PH // 128, 128], BF16)
            EWh = singles.tile([128, 2, PH], BF16)          # ent_emb @ W_head [e-part, et, PH]
            EWt = singles.tile([128, 2, PH], BF16)
            lg_sb = singles.tile([1, PL], F32)
            dbg_zs = singles.tile([128, PH // 128, PL], BF16) if debug else None
            dbg_ctx = singles.tile([128, H // 128, PL], BF16) if debug else None
            dbg_zr = singles.tile([128, 2], F32) if debug else None

            # weight tiles; loads are emitted on the scalar HWDGE queue after
            # the lse exps so they don't steal HBM from the att gathers.
            whb = wpool.tile([128, H // 128, PH], BF16)
            wtb = wpool.tile([128, H // 128, PH], BF16)
            wcb = wpool.tile([128, H // 128, PH], BF16)
            wbb = wpool.tile([128, PH // 128, PH], BF16)
            seqx = wpool.tile([128, S // 128, H], BF16)
            # seqx in 4 chunks so the first lse exp starts early
            for c4 in range(4):
                nc.scalar.dma_start(
                    out=seqx[:, 4 * c4:4 * c4 + 4, :],
                    in_=seqp[:, 4 * c4 * H:(4 * c4 + 4) * H])

            # ---------------- phase 1: att gathers + lse + aggregation ----------
            with tc.tile_pool(name="p1", bufs=1) as p1:
                cnts_t = p1.tile([128, S // 128, E], BF16)
                nc.sync.dma_start(out=cnts_t, in_=cnts[:, :])
                ohm_t = p1.tile([128, NMT, E], F8)
                nc.sync.dma_start(out=ohm_t, in_=ohm[:, :])
                has0b = p1.tile([128, E], F32)
                nc.sync.dma_start(out=has0b, in_=has0r[:, :].to_broadcast([128, E]))
                ag = []
                for mt in range(NMT):
                    g = p1.tile([128, HS], F8, tag=f"ag{mt}")
                    nc.gpsimd.indirect_dma_start(
                        out=g[:, :], out_offset=None, in_=att_k[:, :],
                        in_offset=bass.IndirectOffsetOnAxis(ap=m_off_t[:, mt:mt + 1], axis=0))
                    ag.append(g)

                # logsumexp sums via streaming exp(seq) x counts matmul (no
                # mention gather): sums[h, e] = sum_s exp(seq[s, h]) cnts[s, e]
                with tc.tile_pool(name="ps_l", bufs=1, space="PSUM") as ps_l, \
                     tc.tile_pool(name="expp", bufs=2) as expp:
                    sums_ps = []
                    for hc in range(H // 128):
                        sps = ps_l.tile([128, E], F32, space="PSUM", tag=f"sums{hc}")
                        sums_ps.append(sps)
                    for st in range(S // 128):
                        ex = expp.tile([128, H], BF16, tag="ex")
                        nc.scalar.activation(out=ex[:, :], in_=seqx[:, st, :], func=AF.Exp)
                        for hc in range(H // 128):
                            nc.tensor.matmul(
                                sums_ps[hc][:, :], ex[:, hc * 128:(hc + 1) * 128],
                                cnts_t[:, st, :], start=(st == 0),
                                stop=(st == S // 128 - 1))
                    for hc in range(H // 128):
                        nc.vector.tensor_tensor(out=sums_ps[hc][:, :], in0=sums_ps[hc][:, :],
                                                in1=has0b[:, :], op=OP.add)
                        nc.scalar.activation(out=entTe[:, hc, :], in_=sums_ps[hc][:, :],
                                             func=AF.Ln)
                if debug:
                    nc.sync.dma_start(
                        out=dbg["ent_embT"].rearrange("(t p) e -> p t e", p=128), in_=entTe)

                # weight loads fire once the scalar queue reaches them (post-exp)
                nc.scalar.dma_start(out=whb, in_=w_head[:, :])
                nc.scalar.dma_start(out=wtb, in_=w_tail[:, :])
                nc.scalar.dma_start(out=wcb, in_=w_ctx[:, :])
                nc.scalar.dma_start(out=wbb, in_=w_bil[:, :])

                # entity attention aggregation (one-hot matmuls, windowed)
                with tc.tile_pool(name="ps_a", bufs=1, space="PSUM") as ps_a:
                    for hg in range(2):
                        pas0 = ps_a.tile([128, 8 * SL], F32, space="PSUM", tag="agg0")
                        pas1 = ps_a.tile([128, 8 * SL], F32, space="PSUM", tag="agg1")
                        pas = {0: pas0, 1: pas1}
                        for et in range(2):
                            if not et_mts[et]:
                                nc.vector.memset(pas[et][:, :], 0.0)
                        for mt in range(NMT):
                            for et in mt_ets[mt]:
                                for nch in range(4):  # 2048 = 4 x 512
                                    nc.tensor.matmul(
                                        pas[et][:, nch * 512:(nch + 1) * 512],
                                        ohm_t[:, mt, et * 128:(et + 1) * 128],
                                        ag[mt][:, hg * 2048 + nch * 512:
                                               hg * 2048 + (nch + 1) * 512],
                                        start=(mt == et_mts[et][0]),
                                        stop=(mt == et_mts[et][-1]))
                        for et in range(2):
                            nc.scalar.copy(
                                out=entA_sb[:, et, hg * 2048:(hg + 1) * 2048],
                                in_=pas[et][:, :])
                            nc.sync.dma_start(
                                out=entA_dram.rearrange("(t p) w -> p t w", p=128)[
                                    :, et, hg * 2048:(hg + 1) * 2048],
                                in_=entA_sb[:, et, hg * 2048:(hg + 1) * 2048])
                            if debug:
                                eAb = p1.tile([128, 8 * SL], BF16, tag="entA_dbg")
                                nc.vector.tensor_copy(out=eAb[:, :], in_=pas[et][:, :])
                                nc.sync.dma_start(
                                    out=dbg["entA"].rearrange("(t p) w -> p t w", p=128)[
                                        :, et, hg * 2048:(hg + 1) * 2048],
                                    in_=eAb[:, :])

            # ---------------- phase 2: pair products ----------------
            # evens (tiles 0,2,..,14 = first 128 pairs of each dest core) first
            # so AllToAll #A can fire while the odds still stream. Head rows
            # come from PE one-hot matmuls out of SBUF for 12/16 tiles (pairs
            # sorted by head; Q7 descgen is the limiting serial resource);
            # tails and 4 odd heads are Q7 gathers casting fp8->bf16.
            def pair_tile(pt, pg, prod, ps_r):
                th = pg.tile([128, HS], BF16, tag="th")
                nc.gpsimd.indirect_dma_start(
                    out=th[:, :], out_offset=None, in_=entA_dram[:, :],
                    in_offset=bass.IndirectOffsetOnAxis(
                        ap=p_off_t[:, 2 * pt:2 * pt + 1], axis=0))
                tt = pg.tile([128, HS], BF16, tag="tt")
                nc.gpsimd.indirect_dma_start(
                    out=tt[:, :], out_offset=None, in_=entA_dram[:, :],
                    in_offset=bass.IndirectOffsetOnAxis(
                        ap=p_off_t[:, 2 * pt + 1:2 * pt + 2], axis=0))
                pr = prod.tile([128, HS], BF16, tag="pr")
                nc.vector.tensor_tensor(out=pr[:, :], in0=th[:, :], in1=tt[:, :],
                                        op=OP.mult)
                # fold 16 heads -> 8 on DVE; remaining 8 fold inside the
                # transpose-accumulate matmuls (x identity) on PE.
                nc.vector.tensor_tensor(out=pr[:, :8 * SL], in0=pr[:, :8 * SL],
                                        in1=pr[:, 8 * SL:], op=OP.add)
                rp = ps_r.tile([128, 2, 128], F32, space="PSUM", tag="rp")
                for sh in range(2):
                    for hb in range(8):
                        nc.tensor.matmul(
                            rp[:, sh, :],
                            pr[:, hb * SL + sh * 128: hb * SL + sh * 128 + 128],
                            ident[:, :], start=(hb == 0), stop=(hb == 7))
                    nc.scalar.copy(out=rawT[:, sh, pt, :], in_=rp[:, sh, :])
                c, odd = pt // 2, pt % 2
                nc.sync.dma_start(
                    out=a2a_in[odd][c].rearrange("(sh sp) p -> sp sh p", sh=2),
                    in_=rawT[:, :, pt, :])

            with tc.tile_pool(name="pg", bufs=2) as pg, \
                 tc.tile_pool(name="prod", bufs=2) as prod, \
                 tc.tile_pool(name="ps_r", bufs=3, space="PSUM") as ps_r, \
                 tc.tile_pool(name="ps_e", bufs=1, space="PSUM") as ps_e:
                for c in range(NC):
                    pair_tile(2 * c, pg, prod, ps_r)
                # a2a #A fires as soon as the evens are staged; odds still run.
                nc.gpsimd.collective_compute(
                    "AllToAll", OP.bypass, replica_groups=[list(range(NC))],
                    ins=[a2a_in[0][:, :, :]], outs=[a2a_out[0][:, :, :]])
                nc.sync.dma_start(
                    out=paT[:, :, 0:128],
                    in_=a2a_out[0].rearrange("j (sh sp) q -> sp (j sh) q", sh=2))
                for c in range(NC):
                    pair_tile(2 * c + 1, pg, prod, ps_r)
                nc.gpsimd.collective_compute(
                    "AllToAll", OP.bypass, replica_groups=[list(range(NC))],
                    ins=[a2a_in[1][:, :, :]], outs=[a2a_out[1][:, :, :]])
                nc.sync.dma_start(
                    out=paT[:, :, 128:256],
                    in_=a2a_out[1].rearrange("j (sh sp) q -> sp (j sh) q", sh=2))
                if debug:
                    nc.sync.dma_start(
                        out=dbg["rawT"][:, :],
                        in_=rawT.rearrange("p a b c -> p (a b c)"))

                # EW = ent_emb @ W on PE while a2a #A completes
                for (wsb, dstw) in ((whb, EWh), (wtb, EWt)):
                    for et in range(2):
                        ep = ps_e.tile([128, PH], F32, space="PSUM", tag="ew")
                        for kt in range(H // 128):
                            for nch in range(2):
                                nc.tensor.matmul(
                                    ep[:, nch * 512:(nch + 1) * 512],
                                    entTe[:, kt, et * 128:(et + 1) * 128],
                                    wsb[:, kt, nch * 512:(nch + 1) * 512],
                                    start=(kt == 0), stop=(kt == H // 128 - 1))
                        nc.scalar.copy(out=dstw[:, et, :], in_=ep[:, :])

            # ---------------- phases 4..6: pair-half tails ----------------
            with tc.tile_pool(name="psA", bufs=1, space="PSUM") as psA, \
                 tc.tile_pool(name="psB", bufs=2, space="PSUM") as psB, \
                 tc.tile_pool(name="zscr", bufs=2) as zscr:
                for hf in range(2):
                    q0, q1 = hf * 128, hf * 128 + 128

                    # Z (pairs on partitions) then unnormalized contexts [p, h]
                    zp2 = psA.tile([128, 1], F32, space="PSUM", tag="z2")
                    for t in range(S // 128):
                        nc.tensor.matmul(
                            zp2[:, :], paT[:, t, q0:q1], ones_col[:, :],
                            start=(t == 0), stop=(t == S // 128 - 1))
                    zr = zscr.tile([128, 1], F32, tag="zr")
                    nc.vector.tensor_scalar_add(out=zr[:, :], in0=zp2[:, :],
                                                scalar1=1e-6)
                    nc.vector.reciprocal(out=zr[:, :], in_=zr[:, :])
                    if debug:
                        nc.vector.tensor_copy(out=dbg_zr[:, hf:hf + 1], in_=zr[:, :])
                    ucp = psA.tile([128, H], F32, space="PSUM", tag="ucp")
                    for t in range(S // 128):
                        for nchu in range(2):
                            nc.tensor.matmul(
                                ucp[:, nchu * 512:(nchu + 1) * 512],
                                paT[:, t, q0:q1],
                                seqx[:, t, nchu * 512:(nchu + 1) * 512],
                                start=(t == 0), stop=(t == S // 128 - 1))
                    # normalize on scalar (per-partition scale), transpose back
                    nc.scalar.activation(out=ctxp_sb[:, :], in_=ucp[:, :],
                                         func=AF.Copy, scale=zr[:, :])
                    for mc in range(H // 128):
                        tw = psB.tile([128, 128], F32, space="PSUM", tag="work")
                        nc.tensor.matmul(tw[:, :], ctxp_sb[:, mc * 128:(mc + 1) * 128],
                                         ident[:, :], start=True, stop=True)
                        nc.scalar.copy(out=ctxT[:, mc, :], in_=tw[:, :])
                        if debug:
                            nc.vector.tensor_copy(out=dbg_ctx[:, mc, q0:q1],
                                                  in_=ctxT[:, mc, :])

                    # ---- z_s, z_o (ctx matmul shared) ----
                    for jt in range(PH // 128):
                        cps = psB.tile([128, 128], F32, space="PSUM", tag="work")
                        for kt in range(H // 128):
                            nc.tensor.matmul(
                                cps[:, :], wcb[:, kt, jt * 128:(jt + 1) * 128],
                                ctxT[:, kt, :], start=(kt == 0),
                                stop=(kt == H // 128 - 1))
                        cpsb = zscr.tile([128, 128], BF16, tag="cpsb")
                        nc.scalar.copy(out=cpsb[:, :], in_=cps[:, :])
                        for (ew, oh, bias, dstz) in ((EWh, ohh_t, bhs_t, zsT),
                                                     (EWt, oht_t, bts_t, zoT)):
                            zps = psB.tile([128, 128], F32, space="PSUM", tag="zps")
                            for et in range(2):
                                nc.tensor.matmul(
                                    zps[:, :], ew[:, et, jt * 128:(jt + 1) * 128],
                                    oh[:, et, q0:q1], start=(et == 0), stop=(et == 1))
                            nc.vector.tensor_tensor(out=zps[:, :], in0=zps[:, :],
                                                    in1=cpsb[:, :], op=OP.add)
                            nc.scalar.activation(out=dstz[:, jt, :], in_=zps[:, :],
                                                 func=AF.Tanh, bias=bias[:, jt:jt + 1])
                        if debug:
                            nc.vector.tensor_copy(out=dbg_zs[:, jt, q0:q1],
                                                  in_=zsT[:, jt, :])

                    # ---- bilinear ----
                    lg = psA.tile([1, 128], F32, space="PSUM", tag="lg")
                    for jt in range(PH // 128):
                        ups = psB.tile([128, 128], F32, space="PSUM", tag="work")
                        for it in range(PH // 128):
                            nc.tensor.matmul(
                                ups[:, :], wbb[:, it, jt * 128:(jt + 1) * 128],
                                zsT[:, it, :], start=(it == 0),
                                stop=(it == PH // 128 - 1))
                        pb = zscr.tile([128, 128], BF16, tag="pb")
                        nc.vector.tensor_tensor(out=pb[:, :], in0=ups[:, :],
                                                in1=zoT[:, jt, :], op=OP.mult)
                        nc.tensor.matmul(
                            lg[:, :], ones_col[:, :], pb[:, :],
                            start=(jt == 0), stop=(jt == PH // 128 - 1))
                    nc.vector.tensor_scalar_add(out=lg_sb[:, q0:q1], in0=lg[:, :],
                                                scalar1=bbil_t[:, 0:1])
                if debug:
                    nc.sync.dma_start(
                        out=dbg["ctxnT"].rearrange("(t p) q -> p t q", p=128),
                        in_=dbg_ctx)
                    nc.sync.dma_start(out=dbg["zrec"][:, :], in_=dbg_zr)
                    nc.sync.dma_start(
                        out=dbg["zsT"].rearrange("(t p) q -> p t q", p=128), in_=dbg_zs)
                nc.sync.dma_start(out=out[:, :], in_=lg_sb)

    nc.finalize()
    return nc


def _get_nc(mt_ets, debug=False):
    key = ("nc", mt_ets, debug)
    if key not in _CACHE:
        _CACHE[key] = _build(mt_ets, debug)
    return _CACHE[key]


def _prep_in_maps(inputs):
    import ml_dtypes
    bf16 = ml_dtypes.bfloat16
    f8 = ml_dtypes.float8_e4m3

    att = np.asarray(inputs["attention"], np.float32)          # [16, 2048, 2048]
    seq = np.asarray(inputs["sequence_output"], np.float32)
    mention_idx = np.asarray(inputs["mention_idx"], np.int32)  # [1024]
    entity_ids = np.asarray(inputs["entity_ids"], np.int32)    # [1024]
    pair_h = np.asarray(inputs["pair_h"], np.int32)            # [2048]
    pair_t = np.asarray(inputs["pair_t"], np.int32)

    def pm(x, t):
        """[(t*128), f...] -> partition-major [128, t*f] contiguous rows."""
        f = x.size // (t * 128)
        return np.ascontiguousarray(
            x.reshape(t, 128, f).transpose(1, 0, 2)).reshape(128, t * f)

    counts = np.bincount(entity_ids, minlength=E).astype(np.float32)
    inv_cnt = 1.0 / np.maximum(counts, 1.0)

    cnts_np = np.zeros((S, E), np.float32)
    np.add.at(cnts_np, (mention_idx, entity_ids), 1.0)
    ohm = np.zeros((NM, E), np.float32)
    ohm[np.arange(NM), entity_ids] = inv_cnt[entity_ids]
    has0r = (counts == 0).astype(np.float32)[None, :]

    # which entity-128-halves each mention tile touches (all-zero slabs skipped)
    mt_ets = tuple(
        tuple(sorted(set((entity_ids[mt * 128:(mt + 1) * 128] // 128).tolist())))
        for mt in range(NMT))

    m_off = mention_idx.reshape(NMT, 128).T.copy()             # [128, 8]

    order = np.argsort(pair_h, kind="stable")
    sph = pair_h[order]
    spt = pair_t[order]
    p_off = np.zeros((128, 2 * NPT), np.int32)
    for pt in range(NPT):
        seg = slice(pt * 128, (pt + 1) * 128)
        p_off[:, 2 * pt] = sph[seg]
        p_off[:, 2 * pt + 1] = spt[seg]

    att8 = att.astype(f8)                                      # [16, 2048, 2048]

    shared = {
        "seqp": pm(seq.astype(bf16), S // 128),
        "m_off": m_off,
        "p_off": p_off,
        "cnts": pm(cnts_np.astype(bf16), S // 128),
        "ohm": pm(ohm.astype(f8), NMT),
        "has0r": has0r,
        "w_head": pm(np.asarray(inputs["W_head"], np.float32).astype(bf16), H // 128),
        "w_tail": pm(np.asarray(inputs["W_tail"], np.float32).astype(bf16), H // 128),
        "w_ctx": pm(np.asarray(inputs["W_ctx"], np.float32).astype(bf16), H // 128),
        "w_bil": pm(np.asarray(inputs["W_bil"], np.float32).astype(bf16), PH // 128),
        "b_head": np.asarray(inputs["b_head"], np.float32).reshape(PH // 128, 128).T.copy(),
        "b_tail": np.asarray(inputs["b_tail"], np.float32).reshape(PH // 128, 128).T.copy(),
        "b_bil": np.asarray(inputs["b_bil"], np.float32).reshape(1, 1),
    }

    in_maps = []
    for k in range(NC):
        sk = k * SL
        att_kk = np.ascontiguousarray(
            att8[:, :, sk:sk + SL].transpose(1, 0, 2)).reshape(S, HS)
        ohh_kk = np.zeros((E, PL), np.float32)
        ohh_kk[sph[k * PL:(k + 1) * PL], np.arange(PL)] = 1.0
        oht_kk = np.zeros((E, PL), np.float32)
        oht_kk[spt[k * PL:(k + 1) * PL], np.arange(PL)] = 1.0
        m = dict(shared)
        m["att_k"] = att_kk
        m["ohh_k"] = pm(ohh_kk.astype(bf16), 2)
        m["oht_k"] = pm(oht_kk.astype(bf16), 2)
        in_maps.append(m)
    return in_maps, mt_ets


def _run(inputs, trace=False, debug=False):
    _ensure_axon_profile_hook()
    from concourse.bass_utils import run_bass_kernel_spmd
    in_maps, mt_ets = _prep_in_maps(inputs)
    nc = _get_nc(mt_ets, debug)
    res = run_bass_kernel_spmd(nc, in_maps, list(range(NC)), trace=trace)
    sorted_logits = np.concatenate([np.asarray(res.results[k]["out"][0], np.float32)
                                    for k in range(NC)])
    order = np.argsort(np.asarray(inputs["pair_h"], np.int32), kind="stable")
    logits = np.empty(P, np.float32)
    logits[order] = sorted_logits
    return logits, res


def kernel(**inputs) -> np.ndarray:
    logits, _ = _run(inputs, trace=False)
    return logits


def kernel_traced(**inputs):
    logits, res = _run(inputs, trace=True)
    return logits, res


def kernel_debug(**inputs):
    logits, res = _run(inputs, trace=False, debug=True)
    return logits, res
